# revision 22
# baseline (speedup 1.0000x reference)
"""Trainium2 kernel for nn_RandomizedPruningMasks (scatter + linear).

Computes: w_mod = weight.reshape(-1).at[flip_idx].set(values * 0.1);
          y = x @ w_mod.T            # [B, I] x [O, I] -> [B, O]

Strategy (8 NeuronCores, SPMD):
  - Shard weight along output dim O: core c owns rows [c*OS, (c+1)*OS).
  - The scatter is folded into the streamed weight on the host.
  - One interleaved stream tensor st[P, NI, OS+B]: per I-itile the
    weight slice wT[P, OS] then the x slice xT[P, B]; 1536B contiguous
    per partition per itile keeps DMA descriptors at full rate.
  - Everything streams in fp16 (gate is scale-relative absmax ~2e-2;
    f16 keeps it ~3e-4): per core 6.3MB in + 0.26MB out; DMA floor
    ~15.4us at the measured dual-queue 410GB/s vs f16 PE floor ~14us.
  - Segment schedule: itiles 0..3 as single-itile dma_starts, then
    2-itile segments, alternating the two HWDGE rings (one queue alone
    caps at ~244GB/s).  Small head segments start the PE chase ~1us
    after DGE regardless of which queue dispatches first (the queue
    start order is random with ~1.4us skew); per-ring order is
    preserved so the chase self-heals.
  - 2 matmuls of N=512 per itile (per-instruction stationary reload is
    NOT free: N=256 variants measured ~20% more PE busy).
  - PE warmup (KWARM dummy matmuls on a memset tile) burns the
    0.65/1.2GHz p-state ramp during the DGE/transfer lead-in.
  - Per-core y_c = [B, OS] f16; host concatenates along the output dim.
"""

import os

import numpy as np

import concourse.mybir as mybir
import concourse.tile as tile
from concourse import bacc
from concourse.bass_utils import run_bass_kernel_spmd

N_CORES = 8
P = 128
VALUE_SCALE = 0.1

SEGS = [int(s) for s in os.environ.get(
    'KSEGS', '1,1,1,1,2,2,2,2,2,2,2,2,2,2,2,2,2,2').split(',')]
# Warmup dummy matmuls: sized so the PE (a) burns its 0.65/1.2GHz
# p-state ramp AND (b) starts real work ~3us behind the stream, so it
# never catches the (slightly slower) DMA — a PE that chases stalls in
# micro-gaps, and every micro-gap resets the p-state ramp, pinning it
# at ~1.2-1.3GHz.  Trailing ~3us behind runs gap-free at 2.4GHz.
KWARM = int(os.environ.get('KWARM', '22'))

TRACE = False
_TRACE_KW = {}

DT = mybir.dt.float16


def _build_program(O, I, B):
    OS = O // N_CORES
    NI = I // P
    n_btiles = B // P
    C = OS + B
    assert B % P == 0 and I % P == 0 and sum(SEGS) == NI
    bounds = np.concatenate([[0], np.cumsum(SEGS)]).astype(int)

    nc = bacc.Bacc("TRN2", target_bir_lowering=False, debug=False,
                   num_devices=N_CORES)

    st = nc.declare_dram_parameter("st", [P, NI * C], DT, isOutput=False)
    y = nc.declare_dram_parameter("y", [B, OS], DT, isOutput=True)

    with tile.TileContext(nc) as tc:
        with (
            tc.tile_pool(name="stp", bufs=1) as stp,
            tc.tile_pool(name="wp", bufs=1) as wp,
            tc.tile_pool(name="yp", bufs=1) as yp,
            tc.tile_pool(name="psum", bufs=1, space="PSUM") as psp,
        ):
            t_s = stp.tile([P, NI, C], DT, tag="st")
            t_ps = [psp.tile([P, OS], mybir.dt.float32, tag=f"ps{j}",
                             name=f"ps{j}")
                    for j in range(n_btiles)]

            st_v = st[:].rearrange("p (n c) -> p n c", c=C)
            rings = [nc.sync, nc.scalar]
            # ring assignment: the scalar queue starts ~1.4-2us after
            # sync (random skew), so sync alone carries the first 4
            # itiles (the PE's ramp-up diet); scalar's first segment is
            # it4-5, whose deadline is ~3us later.  Totals stay 16/16.
            # Within a ring itiles are increasing, so landings are
            # in-order per ring and the PE chase self-heals.
            nseg = len(SEGS)
            for g in range(nseg):
                k0, k1 = int(bounds[g]), int(bounds[g + 1])
                if k1 <= 4 or (g % 2 == 1 and g != nseg - 1):
                    ring = rings[0]          # sync
                else:
                    ring = rings[1]          # scalar
                ring.dma_start(out=t_s[:, k0:k1, :],
                               in_=st_v[:, k0:k1, :])

            if KWARM:
                # p-state warmup: short dummy matmuls with no DMA dep
                # keep the PE continuously busy from program start, so
                # the 0.65/1.2GHz ramp elapses before real data lands.
                t_wm = wp.tile([P, P], DT, tag="wm")
                nc.vector.memset(t_wm[:], 0.0)
                for _ in range(KWARM):
                    nc.tensor.matmul(out=t_ps[0][:, 0:P], lhsT=t_wm[:],
                                     rhs=t_wm[:], start=True, stop=True)

            for it in range(NI):
                for j in range(n_btiles):
                    nc.tensor.matmul(
                        out=t_ps[j][:],
                        lhsT=t_s[:, it, OS + j * P:OS + (j + 1) * P],
                        rhs=t_s[:, it, 0:OS],
                        start=(it == 0),
                        stop=(it == NI - 1),
                    )

            # epilogue on DVE only (Act would pull a 1.3us
            # ACT_TABLE_LOAD into the scalar queue at stream start);
            # DMA cannot read PSUM, so cast PSUM->SBUF f16 then store
            for j in range(n_btiles):
                t_y = yp.tile([P, OS], DT, tag=f"y{j}", name=f"y{j}")
                nc.vector.tensor_copy(t_y[:], t_ps[j][:])
                rings[j % 2].dma_start(out=y[j * P:(j + 1) * P, :],
                                       in_=t_y[:])

    nc.compile()
    return nc


def _prep_inputs(x, weight, flip_idx, values):
    """Host-side sharding: per-core [P, NI, OS+B] (wT|xT) stream."""
    O, I = weight.shape
    B = x.shape[0]
    OS = O // N_CORES
    NI = I // P
    np_dt = mybir.dt.np(DT)

    # apply the scatter on host in f32 (last write wins, matching the
    # reference's .at[].set), then round once to the stream dtype
    wf = weight.astype(np.float32).reshape(-1).copy()
    wf[np.asarray(flip_idx)] = (np.asarray(values, np.float32)
                                * np.float32(VALUE_SCALE))
    w_mod = wf.reshape(O, I)

    # xT tile layout: [it, p, b] = x[b, it*P + p]
    xt = x.T.astype(np.float32).reshape(NI, P, B)

    in_maps = []
    for ci in range(N_CORES):
        wT = w_mod[ci * OS:(ci + 1) * OS].T.reshape(NI, P, OS)
        stream = np.concatenate([wT, xt], axis=2)       # [NI, P, OS+B]
        stream = np.ascontiguousarray(
            stream.transpose(1, 0, 2)).reshape(P, NI * (OS + B))
        in_maps.append({"st": stream.astype(np_dt)})

    return in_maps, (O, I, B)


def kernel(x, weight, flip_idx, values):
    x = np.asarray(x)
    weight = np.asarray(weight)
    in_maps, (O, I, B) = _prep_inputs(x, weight, flip_idx, values)
    nc = _build_program(O, I, B)
    res = run_bass_kernel_spmd(nc, in_maps, list(range(N_CORES)),
                               trace=TRACE, **_TRACE_KW)
    if TRACE:
        kernel.last_result = res
    y = np.concatenate([np.asarray(res.results[c]["y"], dtype=np.float32)
                        for c in range(N_CORES)], axis=1)
    return y.astype(np.float32)


# revision 23
# speedup vs baseline: 1.1048x; 1.1048x over previous
"""Trainium2 kernel for nn_RandomizedPruningMasks (scatter + linear).

Computes: w_mod = weight.reshape(-1).at[flip_idx].set(values * 0.1);
          y = x @ w_mod.T            # [B, I] x [O, I] -> [B, O]

Strategy (8 NeuronCores, SPMD):
  - Shard weight along output dim O: core c owns rows [c*OS, (c+1)*OS).
  - The scatter is folded into the streamed weight on the host.
  - One interleaved stream tensor st[P, NI, OS+B]: per I-itile the
    weight slice wT[P, OS] then the x slice xT[P, B]; 1536B contiguous
    per partition per itile keeps DMA descriptors at full rate.
  - Everything streams in fp16 (gate is scale-relative absmax ~2e-2;
    f16 keeps it ~3e-4): per core 6.3MB in + 0.26MB out; DMA floor
    ~15.4us at the measured dual-queue 410GB/s vs f16 PE floor ~14us.
  - Segment schedule: itiles 0..3 as single-itile dma_starts, then
    2-itile segments, alternating the two HWDGE rings (one queue alone
    caps at ~244GB/s).  Small head segments start the PE chase ~1us
    after DGE regardless of which queue dispatches first (the queue
    start order is random with ~1.4us skew); per-ring order is
    preserved so the chase self-heals.
  - 2 matmuls of N=512 per itile (per-instruction stationary reload is
    NOT free: N=256 variants measured ~20% more PE busy).
  - PE warmup (KWARM dummy matmuls on a memset tile) burns the
    0.65/1.2GHz p-state ramp during the DGE/transfer lead-in.
  - Per-core y_c = [B, OS] f16; host concatenates along the output dim.
"""

import os

import numpy as np

import concourse.mybir as mybir
import concourse.tile as tile
from concourse import bacc
from concourse.bass_utils import run_bass_kernel_spmd

N_CORES = 8
P = 128
VALUE_SCALE = 0.1

SEGS = [int(s) for s in os.environ.get(
    'KSEGS', '1,1,1,1,2,2,2,2,2,2,2,2,2,2,2,2,2,2').split(',')]
# Warmup dummy matmuls: sized so the PE (a) burns its 0.65/1.2GHz
# p-state ramp AND (b) starts real work ~3us behind the stream, so it
# never catches the (slightly slower) DMA — a PE that chases stalls in
# micro-gaps, and every micro-gap resets the p-state ramp, pinning it
# at ~1.2-1.3GHz.  Trailing ~3us behind runs gap-free at 2.4GHz.
KWARM = int(os.environ.get('KWARM', '22'))

TRACE = False
_TRACE_KW = {}

DT = mybir.dt.float16


def _build_program(O, I, B):
    OS = O // N_CORES
    NI = I // P
    n_btiles = B // P
    C = OS + B
    assert B % P == 0 and I % P == 0 and sum(SEGS) == NI
    bounds = np.concatenate([[0], np.cumsum(SEGS)]).astype(int)

    nc = bacc.Bacc("TRN2", target_bir_lowering=False, debug=False,
                   num_devices=N_CORES)

    st = nc.declare_dram_parameter("st", [P, NI * C], DT, isOutput=False)
    y = nc.declare_dram_parameter("y", [B, OS], DT, isOutput=True)

    with tile.TileContext(nc) as tc:
        with (
            tc.tile_pool(name="stp", bufs=1) as stp,
            tc.tile_pool(name="wp", bufs=1) as wp,
            tc.tile_pool(name="yp", bufs=1) as yp,
            tc.tile_pool(name="psum", bufs=1, space="PSUM") as psp,
        ):
            t_s = stp.tile([P, NI, C], DT, tag="st")
            t_ps = [psp.tile([P, OS], mybir.dt.float32, tag=f"ps{j}",
                             name=f"ps{j}")
                    for j in range(n_btiles)]

            st_v = st[:].rearrange("p (n c) -> p n c", c=C)
            rings = [nc.sync, nc.scalar]
            # ring assignment: the scalar queue starts ~1.4-2us after
            # sync (random skew), so sync alone carries the first 4
            # itiles (the PE's ramp-up diet); scalar's first segment is
            # it4-5, whose deadline is ~3us later.  Totals stay 16/16.
            # Within a ring itiles are increasing, so landings are
            # in-order per ring and the PE chase self-heals.
            for g in range(len(SEGS)):
                k0, k1 = int(bounds[g]), int(bounds[g + 1])
                rings[g % 2].dma_start(out=t_s[:, k0:k1, :],
                                       in_=st_v[:, k0:k1, :])

            if KWARM:
                # p-state warmup: short dummy matmuls with no DMA dep
                # keep the PE continuously busy from program start, so
                # the 0.65/1.2GHz ramp elapses before real data lands.
                t_wm = wp.tile([P, P], DT, tag="wm")
                nc.vector.memset(t_wm[:], 0.0)
                for _ in range(KWARM):
                    nc.tensor.matmul(out=t_ps[0][:, 0:P], lhsT=t_wm[:],
                                     rhs=t_wm[:], start=True, stop=True)

            for it in range(NI):
                for j in range(n_btiles):
                    nc.tensor.matmul(
                        out=t_ps[j][:],
                        lhsT=t_s[:, it, OS + j * P:OS + (j + 1) * P],
                        rhs=t_s[:, it, 0:OS],
                        start=(it == 0),
                        stop=(it == NI - 1),
                    )

            # epilogue on DVE only (Act would pull a 1.3us
            # ACT_TABLE_LOAD into the scalar queue at stream start);
            # DMA cannot read PSUM, so cast PSUM->SBUF f16 then store
            for j in range(n_btiles):
                t_y = yp.tile([P, OS], DT, tag=f"y{j}", name=f"y{j}")
                nc.vector.tensor_copy(t_y[:], t_ps[j][:])
                rings[j % 2].dma_start(out=y[j * P:(j + 1) * P, :],
                                       in_=t_y[:])

    nc.compile()
    return nc


def _prep_inputs(x, weight, flip_idx, values):
    """Host-side sharding: per-core [P, NI, OS+B] (wT|xT) stream."""
    O, I = weight.shape
    B = x.shape[0]
    OS = O // N_CORES
    NI = I // P
    np_dt = mybir.dt.np(DT)

    # apply the scatter on host in f32 (last write wins, matching the
    # reference's .at[].set), then round once to the stream dtype
    wf = weight.astype(np.float32).reshape(-1).copy()
    wf[np.asarray(flip_idx)] = (np.asarray(values, np.float32)
                                * np.float32(VALUE_SCALE))
    w_mod = wf.reshape(O, I)

    # xT tile layout: [it, p, b] = x[b, it*P + p]
    xt = x.T.astype(np.float32).reshape(NI, P, B)

    in_maps = []
    for ci in range(N_CORES):
        wT = w_mod[ci * OS:(ci + 1) * OS].T.reshape(NI, P, OS)
        stream = np.concatenate([wT, xt], axis=2)       # [NI, P, OS+B]
        stream = np.ascontiguousarray(
            stream.transpose(1, 0, 2)).reshape(P, NI * (OS + B))
        in_maps.append({"st": stream.astype(np_dt)})

    return in_maps, (O, I, B)


def kernel(x, weight, flip_idx, values):
    x = np.asarray(x)
    weight = np.asarray(weight)
    in_maps, (O, I, B) = _prep_inputs(x, weight, flip_idx, values)
    nc = _build_program(O, I, B)
    res = run_bass_kernel_spmd(nc, in_maps, list(range(N_CORES)),
                               trace=TRACE, **_TRACE_KW)
    if TRACE:
        kernel.last_result = res
    y = np.concatenate([np.asarray(res.results[c]["y"], dtype=np.float32)
                        for c in range(N_CORES)], axis=1)
    return y.astype(np.float32)


# revision 24
# speedup vs baseline: 1.1462x; 1.0374x over previous
"""Trainium2 kernel for nn_RandomizedPruningMasks (scatter + linear).

Computes: w_mod = weight.reshape(-1).at[flip_idx].set(values * 0.1);
          y = x @ w_mod.T            # [B, I] x [O, I] -> [B, O]

Strategy (8 NeuronCores, SPMD):
  - Shard weight along output dim O: core c owns rows [c*OS, (c+1)*OS).
  - The scatter is folded into the streamed weight on the host.
  - One interleaved stream tensor st[P, NI, OS+B]: per I-itile the
    weight slice wT[P, OS] then the x slice xT[P, B]; 1536B contiguous
    per partition per itile keeps DMA descriptors at full rate.
  - Everything streams in fp16 (gate is scale-relative absmax ~2e-2;
    f16 keeps it ~3e-4): per core 6.3MB in + 0.26MB out; DMA floor
    ~15.4us at the measured dual-queue 410GB/s vs f16 PE floor ~14us.
  - Segment schedule: itiles 0..3 as single-itile dma_starts, then
    2-itile segments, alternating the two HWDGE rings (one queue alone
    caps at ~244GB/s).  Small head segments start the PE chase ~1us
    after DGE regardless of which queue dispatches first (the queue
    start order is random with ~1.4us skew); per-ring order is
    preserved so the chase self-heals.
  - 2 matmuls of N=512 per itile (per-instruction stationary reload is
    NOT free: N=256 variants measured ~20% more PE busy).
  - PE warmup (KWARM dummy matmuls on a memset tile) burns the
    0.65/1.2GHz p-state ramp during the DGE/transfer lead-in.
  - Per-core y_c = [B, OS] f16; host concatenates along the output dim.
"""

import os

import numpy as np

import concourse.mybir as mybir
import concourse.tile as tile
from concourse import bacc
from concourse.bass_utils import run_bass_kernel_spmd

N_CORES = 8
P = 128
VALUE_SCALE = 0.1

SEGS = [int(s) for s in os.environ.get(
    'KSEGS', '1,1,1,1,1,1,1,1,2,2,2,2,2,2,2,2,2,2,2,2').split(',')]
# Warmup dummy matmuls: sized so the PE (a) burns its 0.65/1.2GHz
# p-state ramp AND (b) starts real work ~3us behind the stream, so it
# never catches the (slightly slower) DMA — a PE that chases stalls in
# micro-gaps, and every micro-gap resets the p-state ramp, pinning it
# at ~1.2-1.3GHz.  Trailing ~3us behind runs gap-free at 2.4GHz.
KWARM = int(os.environ.get('KWARM', '22'))

TRACE = False
_TRACE_KW = {}

DT = mybir.dt.float16


def _build_program(O, I, B):
    OS = O // N_CORES
    NI = I // P
    n_btiles = B // P
    C = OS + B
    assert B % P == 0 and I % P == 0 and sum(SEGS) == NI
    bounds = np.concatenate([[0], np.cumsum(SEGS)]).astype(int)

    nc = bacc.Bacc("TRN2", target_bir_lowering=False, debug=False,
                   num_devices=N_CORES)

    st = nc.declare_dram_parameter("st", [P, NI * C], DT, isOutput=False)
    y = nc.declare_dram_parameter("y", [B, OS], DT, isOutput=True)

    with tile.TileContext(nc) as tc:
        with (
            tc.tile_pool(name="stp", bufs=1) as stp,
            tc.tile_pool(name="wp", bufs=1) as wp,
            tc.tile_pool(name="yp", bufs=1) as yp,
            tc.tile_pool(name="psum", bufs=1, space="PSUM") as psp,
        ):
            t_s = stp.tile([P, NI, C], DT, tag="st")
            t_ps = [psp.tile([P, OS], mybir.dt.float32, tag=f"ps{j}",
                             name=f"ps{j}")
                    for j in range(n_btiles)]

            st_v = st[:].rearrange("p (n c) -> p n c", c=C)
            rings = [nc.sync, nc.scalar]
            # ring assignment: the scalar queue starts ~1.4-2us after
            # sync (random skew), so sync alone carries the first 4
            # itiles (the PE's ramp-up diet); scalar's first segment is
            # it4-5, whose deadline is ~3us later.  Totals stay 16/16.
            # Within a ring itiles are increasing, so landings are
            # in-order per ring and the PE chase self-heals.
            for g in range(len(SEGS)):
                k0, k1 = int(bounds[g]), int(bounds[g + 1])
                rings[g % 2].dma_start(out=t_s[:, k0:k1, :],
                                       in_=st_v[:, k0:k1, :])

            if KWARM:
                # p-state warmup: short dummy matmuls with no DMA dep
                # keep the PE continuously busy from program start, so
                # the 0.65/1.2GHz ramp elapses before real data lands.
                t_wm = wp.tile([P, P], DT, tag="wm")
                nc.vector.memset(t_wm[:], 0.0)
                for _ in range(KWARM):
                    nc.tensor.matmul(out=t_ps[0][:, 0:P], lhsT=t_wm[:],
                                     rhs=t_wm[:], start=True, stop=True)

            for it in range(NI):
                for j in range(n_btiles):
                    nc.tensor.matmul(
                        out=t_ps[j][:],
                        lhsT=t_s[:, it, OS + j * P:OS + (j + 1) * P],
                        rhs=t_s[:, it, 0:OS],
                        start=(it == 0),
                        stop=(it == NI - 1),
                    )

            # epilogue on DVE only (Act would pull a 1.3us
            # ACT_TABLE_LOAD into the scalar queue at stream start);
            # DMA cannot read PSUM, so cast PSUM->SBUF f16 then store
            for j in range(n_btiles):
                t_y = yp.tile([P, OS], DT, tag=f"y{j}", name=f"y{j}")
                nc.vector.tensor_copy(t_y[:], t_ps[j][:])
                rings[j % 2].dma_start(out=y[j * P:(j + 1) * P, :],
                                       in_=t_y[:])

    nc.compile()
    return nc


def _prep_inputs(x, weight, flip_idx, values):
    """Host-side sharding: per-core [P, NI, OS+B] (wT|xT) stream."""
    O, I = weight.shape
    B = x.shape[0]
    OS = O // N_CORES
    NI = I // P
    np_dt = mybir.dt.np(DT)

    # apply the scatter on host in f32 (last write wins, matching the
    # reference's .at[].set), then round once to the stream dtype
    wf = weight.astype(np.float32).reshape(-1).copy()
    wf[np.asarray(flip_idx)] = (np.asarray(values, np.float32)
                                * np.float32(VALUE_SCALE))
    w_mod = wf.reshape(O, I)

    # xT tile layout: [it, p, b] = x[b, it*P + p]
    xt = x.T.astype(np.float32).reshape(NI, P, B)

    in_maps = []
    for ci in range(N_CORES):
        wT = w_mod[ci * OS:(ci + 1) * OS].T.reshape(NI, P, OS)
        stream = np.concatenate([wT, xt], axis=2)       # [NI, P, OS+B]
        stream = np.ascontiguousarray(
            stream.transpose(1, 0, 2)).reshape(P, NI * (OS + B))
        in_maps.append({"st": stream.astype(np_dt)})

    return in_maps, (O, I, B)


def kernel(x, weight, flip_idx, values):
    x = np.asarray(x)
    weight = np.asarray(weight)
    in_maps, (O, I, B) = _prep_inputs(x, weight, flip_idx, values)
    nc = _build_program(O, I, B)
    res = run_bass_kernel_spmd(nc, in_maps, list(range(N_CORES)),
                               trace=TRACE, **_TRACE_KW)
    if TRACE:
        kernel.last_result = res
    y = np.concatenate([np.asarray(res.results[c]["y"], dtype=np.float32)
                        for c in range(N_CORES)], axis=1)
    return y.astype(np.float32)
